# revision 49
# baseline (speedup 1.0000x reference)
"""Trainium2 Bass kernel for nn_Contextualizer (sparse_attention).

Per-core computation (data-parallel over batch B=8 across 8 NeuronCores):
    x0, x1 = split(x, 2, axis=-1)            # [N, D] each, N=2048, D=1024
    xn = x0 / sqrt(sum(x0^2, -1) + eps)      # row-normalize
    cosim = xn @ xn.T                        # [N, N], symmetric
    attn = tril(spatial_proj) * cosim
    out = (attn @ x0) * x1

Only lower-triangle tiles are computed (~half the matmul work).  Gram
formulation: G = x0 @ x0.T, with the two 1/norm factors folded into the
existing elementwise stages:
    attnT[m, n] = maskT[m, n] * G[m, n] * s[m]      (mask stage)
    out[n, d]   = (ctx'[n, d] * s[n]) * x1[n, d]    (gating stage)
where s = 1/sqrt(sum x0^2 + eps), maskT = tril(spatial_proj).T (host),
ctx'[n, d] = sum_m attnT[m, n] * x0[m, d].

All device traffic is bf16 (error budget 2e-2; measured ~4e-3).  The
host pre-splits x into x0/x1, and packs both the transposed x0 and the
consumed lower-triangle mask pairs into partition-major streams so every
DMA reads 2-8KB contiguous per partition (1KB strided reads measured
only ~55% of HBM bandwidth).  No on-device transposes, casts, or PSUM
copies; output returns bf16 and is upcast on the host.  matmul2 output
tiles use 1-bank PSUM chains; mask pairs prefetch a full strip ahead.
"""

import numpy as np

B = 8
N = 2048
D = 1024
P = 128
NT = N // P      # 16 row tiles
DK = D // P      # 8 contraction tiles for matmul1
NJ_W = 512       # matmul1 free-dim chunk (n)
NJS = N // NJ_W  # 4
TPS = NJ_W // P  # 4 n-tiles per strip
NPAIRS = sum(2 * nj + 2 for nj in range(NJS))  # 20 mask pair-tiles
EPS = 1e-8

_NC_CACHE = {}


def _build():
    from concourse import bacc, mybir
    from concourse.tile import TileContext

    f32 = mybir.dt.float32
    bf16 = mybir.dt.bfloat16
    f8 = mybir.dt.float8e4
    AF = mybir.ActivationFunctionType
    OP = mybir.AluOpType
    DR = mybir.MatmulPerfMode.DoubleRow

    nc = bacc.Bacc("TRN2", debug=False, num_devices=B)
    x0_ext = nc.declare_dram_parameter("x0", [N, D], bf16, isOutput=False)
    x1_ext = nc.declare_dram_parameter("x1", [N, D], bf16, isOutput=False)
    x0ts_ext = nc.declare_dram_parameter(
        "x0ts", [P, NJS * DK * NJ_W], bf16, isOutput=False
    )
    x0ts8_ext = nc.declare_dram_parameter(
        "x0ts8", [P, NJS * 2 * NJ_W], f8, isOutput=False
    )
    m_ext = nc.declare_dram_parameter(
        "maskTs", [P, NPAIRS * 2 * NJ_W], bf16, isOutput=False
    )
    out_ext = nc.declare_dram_parameter("out", [N, D], bf16, isOutput=True)

    with TileContext(nc) as tc:
        with (
            tc.tile_pool(name="big", bufs=1) as big,
            tc.tile_pool(name="astrip", bufs=2) as astrip,
            tc.tile_pool(name="maskp", bufs=14) as maskp,
            tc.tile_pool(name="gio", bufs=2) as gio,
            tc.tile_pool(name="outp", bufs=2) as outp,
            tc.tile_pool(name="outl", bufs=2) as outl,
            tc.tile_pool(name="pa", bufs=5, space="PSUM") as pa,
            tc.tile_pool(name="pb", bufs=3, space="PSUM") as pb,
        ):
            xb = big.tile([P, NT, D], bf16)        # x0, natural [m, d]
            x0T = big.tile([P, DK, N], bf16)       # x0 transposed: [d, n]
            x0T8 = big.tile([P, 2, N], f8)         # dk 0-1 as e4m3 pairs
            nrm2 = big.tile([P, NT], f32)
            scal = big.tile([P, NT], f32)
            tmpb = big.tile([P, NT], f32)
            # Shared Square-output scratch: only accum_out is consumed, and
            # Squares are serialized on the Scalar queue anyway.
            sqd = big.tile([P, D], f32)

            def load_x0t_chunk(nj, ways=1, q=None):
                """DMA x0T columns [nj*512, (nj+1)*512) from the host-side
                partition-major packs: the e4m3 dk 0-1 pair plane first
                (each chain's DoubleRow matmul consumes it first), then the
                bf16 dk 2-7 planes (contiguous per partition)."""
                n0 = nj * NJ_W
                c0 = nj * DK * NJ_W
                qf = q or nc.sync
                qf.dma_start(
                    x0T8[:, :, n0 : n0 + NJ_W],
                    x0ts8_ext.ap()[:, nj * 2 * NJ_W : (nj + 1) * 2 * NJ_W],
                )
                dkw = (DK - 2) // ways
                for w in range(ways):
                    qw = q or (nc.scalar if w % 2 == 0 else nc.sync)
                    lo_dk = 2 + w * dkw
                    qw.dma_start(
                        x0T[:, lo_dk : lo_dk + dkw, n0 : n0 + NJ_W],
                        x0ts_ext.ap()[
                            :, c0 + lo_dk * NJ_W : c0 + (lo_dk + dkw) * NJ_W
                        ],
                    )

            def load_xb_pair(t0, q):
                q.dma_start(
                    xb[:, t0 : t0 + 2, :],
                    x0_ext.ap()[t0 * P : (t0 + 2) * P, :].rearrange(
                        "(c p) d -> p c d", p=P
                    ),
                )

            def square_pair(t0, defer_recip=False):
                for i in (t0, t0 + 1):
                    nc.scalar.activation(
                        sqd[:], xb[:, i, :], AF.Square,
                        accum_out=nrm2[:, i : i + 1],
                    )
                # scal[:, t0:t0+2] = 1 / sqrt(nrm2 + EPS); DVE reciprocal
                # is the accurate path (scalar-engine Rsqrt is banned).
                sl = slice(t0, t0 + 2)
                nc.gpsimd.tensor_scalar_add(tmpb[:, sl], nrm2[:, sl], EPS)
                nc.scalar.activation(tmpb[:, sl], tmpb[:, sl], AF.Sqrt)
                recip = lambda: nc.vector.reciprocal(scal[:, sl], tmpb[:, sl])
                if defer_recip:
                    return recip
                recip()

            pair_counter = [0]

            def load_mask():
                """Next mask pair-tile from the host-packed stream
                (2KB/partition contiguous); consumption order == pack order."""
                pi = pair_counter[0]
                pair_counter[0] += 1
                mt = maskp.tile([P, 2, NJ_W], bf16, tag="mt")
                dma_q = nc.sync if pi % 2 == 0 else nc.scalar
                w = 2 * NJ_W
                dma_q.dma_start(mt[:], m_ext.ap()[:, pi * w : (pi + 1) * w])
                return mt

            def phase_a(nj, pre, mid_vector=None):
                """attnT strip for n-chunk nj: tiles mi = 0..4nj+3.

                Edge tiles with mi > 4*nj start at column (mi-4*nj)*128 of
                the strip; everything left of that is above the diagonal
                and masked to zero, so it is skipped.  Mask tiles were
                DMA-issued one strip ahead (the `pre` list).
                """
                n0 = nj * NJ_W
                n_mtiles = 4 * nj + 4
                A = astrip.tile([P, NT, NJ_W], bf16, tag="A")
                for mi2 in range(0, n_mtiles, 2):
                    mt = pre.pop(0)
                    if mi2 == 2 and mid_vector:
                        # strip 0: emit the tiles-2/3 reciprocal on the
                        # vector queue only after the first two stt's, so
                        # they aren't head-of-line blocked behind it
                        mid_vector()
                    for c in range(2):
                        mi = mi2 + c
                        lo = max(0, (mi - 4 * nj) * P)
                        pcs = pa.tile([P, NJ_W], f32)
                        # dk 0-1 as a single fp8 DoubleRow matmul (K=256);
                        # the remaining 6 dk planes accumulate in bf16.
                        nc.tensor.matmul(
                            pcs[:, lo:NJ_W],
                            x0T8[:, :, mi * P : (mi + 1) * P],
                            x0T8[:, :, n0 + lo : n0 + NJ_W],
                            start=True,
                            stop=False,
                            perf_mode=DR,
                        )
                        for dk in range(2, DK):
                            nc.tensor.matmul(
                                pcs[:, lo:NJ_W],
                                x0T[:, dk, mi * P : (mi + 1) * P],
                                x0T[:, dk, n0 + lo : n0 + NJ_W],
                                start=False,
                                stop=(dk == DK - 1),
                            )
                        nc.vector.scalar_tensor_tensor(
                            out=A[:, mi, lo:NJ_W],
                            in0=pcs[:, lo:NJ_W],
                            scalar=scal[:, mi : mi + 1],
                            in1=mt[:, c, lo:NJ_W],
                            op0=OP.mult,
                            op1=OP.mult,
                        )
                return A

            def load_x1_quad(nj):
                """Prefetch the 4 gating tiles phase_b(nj) will consume,
                as two contiguous 2-tile transfers."""
                tiles = []
                for h in range(2):
                    n0 = (TPS * nj + 2 * h) * P
                    x1t = gio.tile([P, 2, D], bf16, tag="x1t")
                    q = nc.sync if h % 2 == 0 else nc.scalar
                    q.dma_start(
                        x1t[:],
                        x1_ext.ap()[n0 : n0 + 2 * P, :].rearrange(
                            "(c p) d -> p c d", p=P
                        ),
                    )
                    tiles.append(x1t)
                return tiles

            def phase_b(nj, A, x1ts, last=False):
                """ctx rows for n-tiles 4nj..4nj+3; scale+gate with x1; DMA."""
                op = None
                for sub in range(TPS):
                    ni = TPS * nj + sub
                    off = sub * P
                    x1t = x1ts[sub // 2]
                    if last:
                        ot = outl.tile([P, D], bf16, tag="ot")
                    else:
                        if sub % 2 == 0:
                            op = outp.tile([P, 2, D], bf16, tag="op")
                        ot = op[:, sub % 2, :]
                    final = last and sub == TPS - 1
                    for dc in range(2):
                        pob = pb.tile([P, 512], f32, tag="pob")
                        for mi in range(ni + 1):
                            nc.tensor.matmul(
                                pob[:],
                                A[:, mi, off : off + P],
                                xb[:, mi, dc * 512 : (dc + 1) * 512],
                                start=(mi == 0),
                                stop=(mi == ni),
                            )
                        if final and dc == 1:
                            # very last tile: quarter-granularity stt+store
                            # so the tail drains as the stt progresses
                            for qd in range(2):
                                c0 = dc * 512 + qd * 256
                                nc.vector.scalar_tensor_tensor(
                                    out=ot[:, c0 : c0 + 256],
                                    in0=pob[:, qd * 256 : (qd + 1) * 256],
                                    scalar=scal[:, ni : ni + 1],
                                    in1=x1t[:, sub % 2, c0 : c0 + 256],
                                    op0=OP.mult,
                                    op1=OP.mult,
                                )
                                qo = nc.sync if qd == 0 else nc.scalar
                                qo.dma_start(
                                    out_ext.ap()[
                                        ni * P : (ni + 1) * P, c0 : c0 + 256
                                    ],
                                    ot[:, c0 : c0 + 256],
                                )
                            continue
                        nc.vector.scalar_tensor_tensor(
                            out=ot[:, dc * 512 : (dc + 1) * 512],
                            in0=pob[:],
                            scalar=scal[:, ni : ni + 1],
                            in1=x1t[:, sub % 2, dc * 512 : (dc + 1) * 512],
                            op0=OP.mult,
                            op1=OP.mult,
                        )
                    if final:
                        nc.gpsimd.dma_start(
                            out_ext.ap()[ni * P : (ni + 1) * P, 0:512],
                            ot[:, 0:512],
                        )
                    elif last:
                        # End-game: split stores across three queues so the
                        # final transfers don't serialize behind one ring.
                        nc.sync.dma_start(
                            out_ext.ap()[ni * P : (ni + 1) * P, 0:384],
                            ot[:, 0:384],
                        )
                        nc.scalar.dma_start(
                            out_ext.ap()[ni * P : (ni + 1) * P, 384:768],
                            ot[:, 384:768],
                        )
                        nc.gpsimd.dma_start(
                            out_ext.ap()[ni * P : (ni + 1) * P, 768:D],
                            ot[:, 768:D],
                        )
                    elif sub % 2 == 1:
                        qo = nc.sync if sub == 1 else nc.scalar
                        qo.dma_start(
                            out_ext.ap()[
                                (ni - 1) * P : (ni + 1) * P, :
                            ].rearrange("(c p) d -> p c d", p=P),
                            op[:],
                        )

            # Startup: x0T chunk 0 halves on the HWDGE queues (first-matmul
            # path), xb singles on the otherwise-idle gpsimd queue (stats
            # path), strip-0 masks, then chunk 1.  Each loop body issues the
            # NEXT strip's DMAs first so transfers overlap this strip's PE.
            # Startup, in need-order: x0T chunk 0 pieces (first matmul
            # chain), xb 0-1 (Square->stats->stt path) on the low-latency
            # HWDGE queues, xb 2-3 on gpsimd SWDGE (tolerates cold-start),
            # strip-0 masks, then chunk 1.
            load_x0t_chunk(0, ways=6)
            nc.sync.dma_start(xb[:, 0, :], x0_ext.ap()[0:P, :])
            nc.scalar.dma_start(xb[:, 1, :], x0_ext.ap()[P : 2 * P, :])
            for t in (2, 3):
                nc.gpsimd.dma_start(
                    xb[:, t, :], x0_ext.ap()[t * P : (t + 1) * P, :]
                )
            masks_cur = [load_mask(), load_mask()]
            load_x0t_chunk(1, ways=2)
            square_pair(0)
            recip23 = square_pair(2, defer_recip=True)

            prev_A = None
            for nj in range(NJS):
                x1_cur = load_x1_quad(nj - 1) if prev_A is not None else None
                if nj + 2 < NJS:
                    load_x0t_chunk(
                        nj + 2, q=nc.sync if nj % 2 == 0 else nc.scalar
                    )
                masks_next = None
                if nj + 1 < NJS:
                    load_xb_pair(4 * nj + 4, nc.sync)
                    load_xb_pair(4 * nj + 6, nc.scalar)
                    masks_next = [
                        load_mask() for _ in range(2 * (nj + 1) + 2)
                    ]
                A = phase_a(
                    nj, masks_cur, mid_vector=recip23 if nj == 0 else None
                )
                if prev_A is not None:
                    phase_b(nj - 1, prev_A, x1_cur)
                if nj + 1 < NJS:
                    square_pair(4 * nj + 4)
                    square_pair(4 * nj + 6)
                masks_cur = masks_next
                prev_A = A
            phase_b(NJS - 1, prev_A, load_x1_quad(NJS - 1), last=True)

    nc.compile()
    return nc


def _get_nc():
    if "nc" not in _NC_CACHE:
        _NC_CACHE["nc"] = _build()
    return _NC_CACHE["nc"]


def _prep(x, spatial_proj):
    import ml_dtypes

    bf = ml_dtypes.bfloat16
    x = np.asarray(x, dtype=np.float32)
    sp = np.asarray(spatial_proj, dtype=np.float32)
    x0f = x[:, :, :D]
    x0 = np.ascontiguousarray(x0f).astype(bf)
    x1 = np.ascontiguousarray(x[:, :, D:]).astype(bf)
    # partition-major pack of x0^T: x0ts[b, p, nj*DK*512 + dk*512 + n'] =
    # x0[b, nj*512 + n', dk*128 + p]
    x0ts = np.ascontiguousarray(
        x0f.reshape(B, NJS, NJ_W, DK, P).transpose(0, 4, 1, 3, 2)
    ).reshape(B, P, NJS * DK * NJ_W).astype(bf)
    # e4m3 pack of the dk 0-1 planes for the DoubleRow (K=256) matmul
    x0ts8 = np.ascontiguousarray(
        x0f[:, :, : 2 * P].reshape(B, NJS, NJ_W, 2, P).transpose(0, 4, 1, 3, 2)
    ).reshape(B, P, NJS * 2 * NJ_W).astype(ml_dtypes.float8_e4m3)
    # mask pair stream in consumption order
    maskT = np.tril(sp).T
    pairs = []
    for nj in range(NJS):
        for mi2 in range(0, 4 * nj + 4, 2):
            blk = maskT[
                mi2 * P : (mi2 + 2) * P, nj * NJ_W : (nj + 1) * NJ_W
            ]
            pairs.append(
                blk.reshape(2, P, NJ_W).transpose(1, 0, 2).reshape(P, -1)
            )
    maskTs = np.concatenate(pairs, axis=1).astype(bf)
    return x0, x1, x0ts, x0ts8, maskTs


def _run(x, spatial_proj, trace=False):
    from concourse.bass_utils import run_bass_kernel_spmd

    nc = _get_nc()
    x0, x1, x0ts, x0ts8, maskTs = _prep(x, spatial_proj)
    in_maps = [
        {
            "x0": x0[b],
            "x1": x1[b],
            "x0ts": x0ts[b],
            "x0ts8": x0ts8[b],
            "maskTs": maskTs,
        }
        for b in range(B)
    ]
    res = run_bass_kernel_spmd(
        nc, in_maps, core_ids=list(range(B)), trace=trace
    )
    out = np.stack([res.results[b]["out"] for b in range(B)], axis=0)
    return out.astype(np.float32), res


def kernel(x, spatial_proj):
    out, _ = _run(x, spatial_proj, trace=False)
    return out


if __name__ == "__main__":
    rng = np.random.default_rng(0)
    x = rng.standard_normal((B, N, 2 * D), dtype=np.float32)
    sp = (rng.standard_normal((N, N), dtype=np.float32) * np.sqrt(1.0 / N)).astype(
        np.float32
    )
    out = kernel(x, sp)
    print("out shape", out.shape, out.dtype)


# revision 50
# speedup vs baseline: 1.2491x; 1.2491x over previous
"""Trainium2 Bass kernel for nn_Contextualizer (sparse_attention).

Per-core computation (data-parallel over batch B=8 across 8 NeuronCores):
    x0, x1 = split(x, 2, axis=-1)            # [N, D] each, N=2048, D=1024
    xn = x0 / sqrt(sum(x0^2, -1) + eps)      # row-normalize
    cosim = xn @ xn.T                        # [N, N], symmetric
    attn = tril(spatial_proj) * cosim
    out = (attn @ x0) * x1

Only lower-triangle tiles are computed (~half the matmul work).  Gram
formulation: G = x0 @ x0.T, with the two 1/norm factors folded into the
existing elementwise stages:
    attnT[m, n] = maskT[m, n] * G[m, n] * s[m]      (mask stage)
    out[n, d]   = (ctx'[n, d] * s[n]) * x1[n, d]    (gating stage)
where s = 1/sqrt(sum x0^2 + eps), maskT = tril(spatial_proj).T (host),
ctx'[n, d] = sum_m attnT[m, n] * x0[m, d].

All device traffic is bf16 (error budget 2e-2; measured ~4e-3).  The
host pre-splits x into x0/x1, and packs both the transposed x0 and the
consumed lower-triangle mask pairs into partition-major streams so every
DMA reads 2-8KB contiguous per partition (1KB strided reads measured
only ~55% of HBM bandwidth).  No on-device transposes, casts, or PSUM
copies; output returns bf16 and is upcast on the host.  matmul2 output
tiles use 1-bank PSUM chains; mask pairs prefetch a full strip ahead.
"""

import numpy as np

B = 8
N = 2048
D = 1024
P = 128
NT = N // P      # 16 row tiles
DK = D // P      # 8 contraction tiles for matmul1
NJ_W = 512       # matmul1 free-dim chunk (n)
NJS = N // NJ_W  # 4
TPS = NJ_W // P  # 4 n-tiles per strip
NPAIRS = sum(2 * nj + 2 for nj in range(NJS))  # 20 mask pair-tiles
EPS = 1e-8

_NC_CACHE = {}


def _build():
    from concourse import bacc, mybir
    from concourse.tile import TileContext

    f32 = mybir.dt.float32
    bf16 = mybir.dt.bfloat16
    f8 = mybir.dt.float8e4
    AF = mybir.ActivationFunctionType
    OP = mybir.AluOpType
    DR = mybir.MatmulPerfMode.DoubleRow

    nc = bacc.Bacc("TRN2", debug=False, num_devices=B)
    x0_ext = nc.declare_dram_parameter("x0", [N, D], bf16, isOutput=False)
    x1_ext = nc.declare_dram_parameter("x1", [N, D], bf16, isOutput=False)
    x0ts_ext = nc.declare_dram_parameter(
        "x0ts", [P, NJS * DK * NJ_W], bf16, isOutput=False
    )
    x0ts8_ext = nc.declare_dram_parameter(
        "x0ts8", [P, NJS * 2 * NJ_W], f8, isOutput=False
    )
    m_ext = nc.declare_dram_parameter(
        "maskTs", [P, NPAIRS * 2 * NJ_W], bf16, isOutput=False
    )
    out_ext = nc.declare_dram_parameter("out", [N, D], bf16, isOutput=True)

    with TileContext(nc) as tc:
        with (
            tc.tile_pool(name="big", bufs=1) as big,
            tc.tile_pool(name="astrip", bufs=2) as astrip,
            tc.tile_pool(name="maskp", bufs=14) as maskp,
            tc.tile_pool(name="gio", bufs=2) as gio,
            tc.tile_pool(name="outp", bufs=2) as outp,
            tc.tile_pool(name="outl", bufs=2) as outl,
            tc.tile_pool(name="pa", bufs=5, space="PSUM") as pa,
            tc.tile_pool(name="pb", bufs=3, space="PSUM") as pb,
        ):
            xb = big.tile([P, NT, D], bf16)        # x0, natural [m, d]
            x0T = big.tile([P, DK, N], bf16)       # x0 transposed: [d, n]
            x0T8 = big.tile([P, 2, N], f8)         # dk 0-1 as e4m3 pairs
            nrm2 = big.tile([P, NT], f32)
            scal = big.tile([P, NT], f32)
            tmpb = big.tile([P, NT], f32)
            # Shared Square-output scratch: only accum_out is consumed, and
            # Squares are serialized on the Scalar queue anyway.
            sqd = big.tile([P, D], f32)

            def load_x0t_chunk(nj, ways=1, q=None):
                """DMA x0T columns [nj*512, (nj+1)*512) from the host-side
                partition-major packs: the e4m3 dk 0-1 pair plane first
                (each chain's DoubleRow matmul consumes it first), then the
                bf16 dk 2-7 planes (contiguous per partition)."""
                n0 = nj * NJ_W
                c0 = nj * DK * NJ_W
                qf = q or nc.sync
                qf.dma_start(
                    x0T8[:, :, n0 : n0 + NJ_W],
                    x0ts8_ext.ap()[:, nj * 2 * NJ_W : (nj + 1) * 2 * NJ_W],
                )
                dkw = (DK - 2) // ways
                for w in range(ways):
                    qw = q or (nc.scalar if w % 2 == 0 else nc.sync)
                    lo_dk = 2 + w * dkw
                    qw.dma_start(
                        x0T[:, lo_dk : lo_dk + dkw, n0 : n0 + NJ_W],
                        x0ts_ext.ap()[
                            :, c0 + lo_dk * NJ_W : c0 + (lo_dk + dkw) * NJ_W
                        ],
                    )

            def load_xb_pair(t0, q):
                q.dma_start(
                    xb[:, t0 : t0 + 2, :],
                    x0_ext.ap()[t0 * P : (t0 + 2) * P, :].rearrange(
                        "(c p) d -> p c d", p=P
                    ),
                )

            def square_pair(t0, defer_recip=False):
                for i in (t0, t0 + 1):
                    nc.scalar.activation(
                        sqd[:], xb[:, i, :], AF.Square,
                        accum_out=nrm2[:, i : i + 1],
                    )
                # scal[:, t0:t0+2] = 1 / sqrt(nrm2 + EPS); DVE reciprocal
                # is the accurate path (scalar-engine Rsqrt is banned).
                sl = slice(t0, t0 + 2)
                nc.gpsimd.tensor_scalar_add(tmpb[:, sl], nrm2[:, sl], EPS)
                nc.scalar.activation(tmpb[:, sl], tmpb[:, sl], AF.Sqrt)
                recip = lambda: nc.vector.reciprocal(scal[:, sl], tmpb[:, sl])
                if defer_recip:
                    return recip
                recip()

            pair_counter = [0]

            def load_mask():
                """Next mask pair-tile from the host-packed stream
                (2KB/partition contiguous); consumption order == pack order."""
                pi = pair_counter[0]
                pair_counter[0] += 1
                mt = maskp.tile([P, 2, NJ_W], bf16, tag="mt")
                dma_q = nc.sync if pi % 2 == 0 else nc.scalar
                w = 2 * NJ_W
                dma_q.dma_start(mt[:], m_ext.ap()[:, pi * w : (pi + 1) * w])
                return mt

            def phase_a(nj, pre, mid_vector=None):
                """attnT strip for n-chunk nj: tiles mi = 0..4nj+3.

                Edge tiles with mi > 4*nj start at column (mi-4*nj)*128 of
                the strip; everything left of that is above the diagonal
                and masked to zero, so it is skipped.  Mask tiles were
                DMA-issued one strip ahead (the `pre` list).
                """
                n0 = nj * NJ_W
                n_mtiles = 4 * nj + 4
                A = astrip.tile([P, NT, NJ_W], bf16, tag="A")
                for mi2 in range(0, n_mtiles, 2):
                    mt = pre.pop(0)
                    if mi2 == 2 and mid_vector:
                        # strip 0: emit the tiles-2/3 reciprocal on the
                        # vector queue only after the first two stt's, so
                        # they aren't head-of-line blocked behind it
                        mid_vector()
                    for c in range(2):
                        mi = mi2 + c
                        lo = max(0, (mi - 4 * nj) * P)
                        pcs = pa.tile([P, NJ_W], f32)
                        # dk 0-1 as a single fp8 DoubleRow matmul (K=256);
                        # the remaining 6 dk planes accumulate in bf16.
                        nc.tensor.matmul(
                            pcs[:, lo:NJ_W],
                            x0T8[:, :, mi * P : (mi + 1) * P],
                            x0T8[:, :, n0 + lo : n0 + NJ_W],
                            start=True,
                            stop=False,
                            perf_mode=DR,
                        )
                        for dk in range(2, DK):
                            nc.tensor.matmul(
                                pcs[:, lo:NJ_W],
                                x0T[:, dk, mi * P : (mi + 1) * P],
                                x0T[:, dk, n0 + lo : n0 + NJ_W],
                                start=False,
                                stop=(dk == DK - 1),
                            )
                        nc.vector.scalar_tensor_tensor(
                            out=A[:, mi, lo:NJ_W],
                            in0=pcs[:, lo:NJ_W],
                            scalar=scal[:, mi : mi + 1],
                            in1=mt[:, c, lo:NJ_W],
                            op0=OP.mult,
                            op1=OP.mult,
                        )
                return A

            def load_x1_quad(nj):
                """Prefetch the 4 gating tiles phase_b(nj) will consume,
                as two contiguous 2-tile transfers."""
                tiles = []
                for h in range(2):
                    n0 = (TPS * nj + 2 * h) * P
                    x1t = gio.tile([P, 2, D], bf16, tag="x1t")
                    q = nc.sync if h % 2 == 0 else nc.scalar
                    q.dma_start(
                        x1t[:],
                        x1_ext.ap()[n0 : n0 + 2 * P, :].rearrange(
                            "(c p) d -> p c d", p=P
                        ),
                    )
                    tiles.append(x1t)
                return tiles

            def phase_b(nj, A, x1ts, last=False):
                """ctx rows for n-tiles 4nj..4nj+3; scale+gate with x1; DMA."""
                op = None
                for sub in range(TPS):
                    ni = TPS * nj + sub
                    off = sub * P
                    x1t = x1ts[sub // 2]
                    if last:
                        ot = outl.tile([P, D], bf16, tag="ot")
                    else:
                        if sub % 2 == 0:
                            op = outp.tile([P, 2, D], bf16, tag="op")
                        ot = op[:, sub % 2, :]
                    final = last and sub == TPS - 1
                    for dc in range(2):
                        pob = pb.tile([P, 512], f32, tag="pob")
                        for mi in range(ni + 1):
                            nc.tensor.matmul(
                                pob[:],
                                A[:, mi, off : off + P],
                                xb[:, mi, dc * 512 : (dc + 1) * 512],
                                start=(mi == 0),
                                stop=(mi == ni),
                            )
                        if final and dc == 1:
                            # very last tile: quarter-granularity stt+store
                            # so the tail drains as the stt progresses
                            for qd in range(2):
                                c0 = dc * 512 + qd * 256
                                nc.vector.scalar_tensor_tensor(
                                    out=ot[:, c0 : c0 + 256],
                                    in0=pob[:, qd * 256 : (qd + 1) * 256],
                                    scalar=scal[:, ni : ni + 1],
                                    in1=x1t[:, sub % 2, c0 : c0 + 256],
                                    op0=OP.mult,
                                    op1=OP.mult,
                                )
                                qo = nc.sync if qd == 0 else nc.scalar
                                qo.dma_start(
                                    out_ext.ap()[
                                        ni * P : (ni + 1) * P, c0 : c0 + 256
                                    ],
                                    ot[:, c0 : c0 + 256],
                                )
                            continue
                        nc.vector.scalar_tensor_tensor(
                            out=ot[:, dc * 512 : (dc + 1) * 512],
                            in0=pob[:],
                            scalar=scal[:, ni : ni + 1],
                            in1=x1t[:, sub % 2, dc * 512 : (dc + 1) * 512],
                            op0=OP.mult,
                            op1=OP.mult,
                        )
                    if final:
                        nc.gpsimd.dma_start(
                            out_ext.ap()[ni * P : (ni + 1) * P, 0:512],
                            ot[:, 0:512],
                        )
                    elif last:
                        # End-game: split stores across three queues so the
                        # final transfers don't serialize behind one ring.
                        nc.sync.dma_start(
                            out_ext.ap()[ni * P : (ni + 1) * P, 0:384],
                            ot[:, 0:384],
                        )
                        nc.scalar.dma_start(
                            out_ext.ap()[ni * P : (ni + 1) * P, 384:768],
                            ot[:, 384:768],
                        )
                        nc.gpsimd.dma_start(
                            out_ext.ap()[ni * P : (ni + 1) * P, 768:D],
                            ot[:, 768:D],
                        )
                    elif sub % 2 == 1:
                        qo = nc.sync if sub == 1 else nc.scalar
                        qo.dma_start(
                            out_ext.ap()[
                                (ni - 1) * P : (ni + 1) * P, :
                            ].rearrange("(c p) d -> p c d", p=P),
                            op[:],
                        )

            # Startup: x0T chunk 0 halves on the HWDGE queues (first-matmul
            # path), xb singles on the otherwise-idle gpsimd queue (stats
            # path), strip-0 masks, then chunk 1.  Each loop body issues the
            # NEXT strip's DMAs first so transfers overlap this strip's PE.
            # Startup, in need-order: x0T chunk 0 pieces (first matmul
            # chain), xb 0-1 (Square->stats->stt path) on the low-latency
            # HWDGE queues, xb 2-3 on gpsimd SWDGE (tolerates cold-start),
            # strip-0 masks, then chunk 1.
            load_x0t_chunk(0, ways=3)
            nc.sync.dma_start(xb[:, 0, :], x0_ext.ap()[0:P, :])
            nc.scalar.dma_start(xb[:, 1, :], x0_ext.ap()[P : 2 * P, :])
            for t in (2, 3):
                nc.gpsimd.dma_start(
                    xb[:, t, :], x0_ext.ap()[t * P : (t + 1) * P, :]
                )
            masks_cur = [load_mask(), load_mask()]
            load_x0t_chunk(1, ways=2)
            square_pair(0)
            recip23 = square_pair(2, defer_recip=True)

            prev_A = None
            for nj in range(NJS):
                x1_cur = load_x1_quad(nj - 1) if prev_A is not None else None
                if nj + 2 < NJS:
                    load_x0t_chunk(
                        nj + 2, q=nc.sync if nj % 2 == 0 else nc.scalar
                    )
                masks_next = None
                if nj + 1 < NJS:
                    load_xb_pair(4 * nj + 4, nc.sync)
                    load_xb_pair(4 * nj + 6, nc.scalar)
                    masks_next = [
                        load_mask() for _ in range(2 * (nj + 1) + 2)
                    ]
                A = phase_a(
                    nj, masks_cur, mid_vector=recip23 if nj == 0 else None
                )
                if prev_A is not None:
                    phase_b(nj - 1, prev_A, x1_cur)
                if nj + 1 < NJS:
                    square_pair(4 * nj + 4)
                    square_pair(4 * nj + 6)
                masks_cur = masks_next
                prev_A = A
            phase_b(NJS - 1, prev_A, load_x1_quad(NJS - 1), last=True)

    nc.compile()
    return nc


def _get_nc():
    if "nc" not in _NC_CACHE:
        _NC_CACHE["nc"] = _build()
    return _NC_CACHE["nc"]


def _prep(x, spatial_proj):
    import ml_dtypes

    bf = ml_dtypes.bfloat16
    x = np.asarray(x, dtype=np.float32)
    sp = np.asarray(spatial_proj, dtype=np.float32)
    x0f = x[:, :, :D]
    x0 = np.ascontiguousarray(x0f).astype(bf)
    x1 = np.ascontiguousarray(x[:, :, D:]).astype(bf)
    # partition-major pack of x0^T: x0ts[b, p, nj*DK*512 + dk*512 + n'] =
    # x0[b, nj*512 + n', dk*128 + p]
    x0ts = np.ascontiguousarray(
        x0f.reshape(B, NJS, NJ_W, DK, P).transpose(0, 4, 1, 3, 2)
    ).reshape(B, P, NJS * DK * NJ_W).astype(bf)
    # e4m3 pack of the dk 0-1 planes for the DoubleRow (K=256) matmul
    x0ts8 = np.ascontiguousarray(
        x0f[:, :, : 2 * P].reshape(B, NJS, NJ_W, 2, P).transpose(0, 4, 1, 3, 2)
    ).reshape(B, P, NJS * 2 * NJ_W).astype(ml_dtypes.float8_e4m3)
    # mask pair stream in consumption order
    maskT = np.tril(sp).T
    pairs = []
    for nj in range(NJS):
        for mi2 in range(0, 4 * nj + 4, 2):
            blk = maskT[
                mi2 * P : (mi2 + 2) * P, nj * NJ_W : (nj + 1) * NJ_W
            ]
            pairs.append(
                blk.reshape(2, P, NJ_W).transpose(1, 0, 2).reshape(P, -1)
            )
    maskTs = np.concatenate(pairs, axis=1).astype(bf)
    return x0, x1, x0ts, x0ts8, maskTs


def _run(x, spatial_proj, trace=False):
    from concourse.bass_utils import run_bass_kernel_spmd

    nc = _get_nc()
    x0, x1, x0ts, x0ts8, maskTs = _prep(x, spatial_proj)
    in_maps = [
        {
            "x0": x0[b],
            "x1": x1[b],
            "x0ts": x0ts[b],
            "x0ts8": x0ts8[b],
            "maskTs": maskTs,
        }
        for b in range(B)
    ]
    res = run_bass_kernel_spmd(
        nc, in_maps, core_ids=list(range(B)), trace=trace
    )
    out = np.stack([res.results[b]["out"] for b in range(B)], axis=0)
    return out.astype(np.float32), res


def kernel(x, spatial_proj):
    out, _ = _run(x, spatial_proj, trace=False)
    return out


if __name__ == "__main__":
    rng = np.random.default_rng(0)
    x = rng.standard_normal((B, N, 2 * D), dtype=np.float32)
    sp = (rng.standard_normal((N, N), dtype=np.float32) * np.sqrt(1.0 / N)).astype(
        np.float32
    )
    out = kernel(x, sp)
    print("out shape", out.shape, out.dtype)
